# revision 1
# baseline (speedup 1.0000x reference)
"""Trainium2 Bass kernel for the DifferentiableQuantumCircuit problem.

Math: output = |U x / ||x|| |^2 with U = kron of 12 single-qubit U3 gates
applied twice (2 layers). Gates on different qubits commute, so the two
layers fuse into ONE kron-product unitary with per-qubit gates
G_q = U3_layer2(q) @ U3_layer1(q).

Host side: x is pre-normalized (x / ||x||, numpy) and the gate constants
are built in float64 then cast to f32 (stage 1) / bf16 (stage 2), same
precedent as the original host-side gate construction.

State index split: i = q5 * 128 + l7, with q5 = qubits 0-4 (5 MSBs) and
l7 = qubits 5-11 (7 LSBs, contiguous in memory -> 512B DMA bursts).
U_total = M5a (x) M7b with M5a = kron(G_0..G_4) [32x32] acting on q5 and
M7b = kron(G_5..G_11) [128x128] acting on l7.

Per-core pipeline (512 samples/core, 4 chunks of 128 samples b=(bh,b2),
bh in [0,32), b2 in [0,4); chunks split into 2 halves of 16 bh each,
each half = 4 groups of 4 c-tiles, c-tile = one bh = 4 samples):
  1. DMA-load half: Xh[(b2,q5), (bh,l7)] = x[bh*4+b2, q5*128+l7] (f32;
     gate constants are DMA'd FIRST so compute starts immediately)
  2. stage 1 (PE "trick" matmuls, f32r): stationary = X c-tile,
     moving = [Re(G5bd^T) | Im(G5bd^T)] with G5bd = I4 (x) M5a
     -> psum[l7, (j, re/im, (b2',q5'))] (applies the 5-qubit gate group
     AND transposes l7 onto partitions)
  3. evacuate psum: TWO bf16 casts per group on VectorE, split
     asymmetrically ([0:256] then [256:1024]) so stage-2's first
     matmuls start as early as possible -- the s1->evac->s2 chain
     latency sets the pipeline period via the 2-deep psum buffering
  4. stage 2 (bf16): stationary = S1 re/im chunks, moving =
     [Re(M7b^T)|Im(M7b^T)] / [-Im|Re], accumulating
     -> psum[(b2',q5'), (j, re/im, l7')]
  5. ONE ScalarE Square per group: T12 = psum^2 (bf16)
  6. GpSimd adds re^2 + im^2 -> P (f32; last half splits across
     Vector/GpSimd so the tail drains in parallel)
  7. DMA-store P quarters -> out[b, i] (512B bursts); store issue is
     deferred a few groups so it never head-of-line-blocks a DGE ring,
     all on the sync ring (free once loads are issued)

Emission is software-pipelined at depth 2 (stage-2 of item i emitted
after stage-1 of item i+2) so the in-order Tensor queue never stalls
on the evacuations; LDWEIGHTS overlap in-flight MATMULs via the
engine queue's reorder window.
"""

from contextlib import ExitStack

import ml_dtypes
import numpy as np

import concourse.tile as tile
from concourse import bacc, mybir
from concourse.bass_utils import run_bass_kernel_spmd

F32 = mybir.dt.float32
F32R = mybir.dt.float32r
BF16 = mybir.dt.bfloat16

NUM_QUBITS = 12
D = 4096
B = 4096
N_CORES = 8
B_CORE = B // N_CORES  # 512
CHUNK = 128
N_CHUNKS = B_CORE // CHUNK  # 4
GROUP = 4  # c-tiles per psum group tile (2 banks)
HALF = D // 2  # free columns per half-chunk (16 bh x 128 l7)


def _u3(theta, phi, lam):
    """Single-qubit U3 gate, complex128 [2,2] (same formula as reference)."""
    c = np.cos(theta / 2.0)
    s = np.sin(theta / 2.0)
    return np.array(
        [
            [c, -np.exp(1j * lam) * s],
            [np.exp(1j * phi) * s, np.exp(1j * (phi + lam)) * c],
        ],
        dtype=np.complex128,
    )


def _gate_consts(thetas, phis, lams):
    """Build the constant moving-operand matrices for both PE stages (bf16)."""
    thetas = np.asarray(thetas, dtype=np.float64)
    phis = np.asarray(phis, dtype=np.float64)
    lams = np.asarray(lams, dtype=np.float64)
    gates = []
    for q in range(NUM_QUBITS):
        g1 = _u3(thetas[0, q], phis[0, q], lams[0, q])
        g2 = _u3(thetas[1, q], phis[1, q], lams[1, q])
        gates.append(g2 @ g1)  # layer 1 applied first, then layer 2

    m5a = gates[0]
    for q in range(1, 5):
        m5a = np.kron(m5a, gates[q])  # [32,32], acts on q5 (bits 0-4)
    m7b = gates[5]
    for q in range(6, 12):
        m7b = np.kron(m7b, gates[q])  # [128,128], acts on l7 (bits 5-11)

    g5 = np.kron(np.eye(4), m5a)  # [128,128] block-diag over (b2, q5)

    mv1 = np.concatenate([g5.T.real, g5.T.imag], axis=1)  # [128,256]
    mv2a = np.concatenate([m7b.T.real, m7b.T.imag], axis=1)
    mv2b = np.concatenate([-m7b.T.imag, m7b.T.real], axis=1)
    return (
        np.ascontiguousarray(mv1, dtype=np.float32),
        np.ascontiguousarray(mv2a).astype(ml_dtypes.bfloat16),
        np.ascontiguousarray(mv2b).astype(ml_dtypes.bfloat16),
    )


def _build_nc():
    nc = bacc.Bacc(
        "TRN2", target_bir_lowering=False, debug=False, num_devices=N_CORES
    )
    x_ap = nc.dram_tensor("x", [B_CORE, D], F32R, kind="ExternalInput").ap()
    mv1_ap = nc.dram_tensor("mv1", [128, 256], F32R, kind="ExternalInput").ap()
    mv2a_ap = nc.dram_tensor("mv2a", [128, 256], BF16, kind="ExternalInput").ap()
    mv2b_ap = nc.dram_tensor("mv2b", [128, 256], BF16, kind="ExternalInput").ap()
    out_ap = nc.dram_tensor("probs", [B_CORE, D], F32, kind="ExternalOutput").ap()

    with tile.TileContext(nc) as tc, ExitStack() as ctx:
        # gate constants FIRST on the sync queue so the pipeline can
        # start as soon as the first X pieces land
        consts = ctx.enter_context(tc.tile_pool(name="consts", bufs=1))
        mv1_tt = consts.tile([128, 256], F32R, tag="mv1")
        nc.sync.dma_start(mv1_tt[:], mv1_ap[:])
        mv2a_tt = consts.tile([128, 256], BF16, tag="mv2a")
        nc.sync.dma_start(mv2a_tt[:], mv2a_ap[:])
        mv2b_tt = consts.tile([128, 256], BF16, tag="mv2b")
        nc.sync.dma_start(mv2b_tt[:], mv2b_ap[:])
        mv1_t = mv1_tt[:]
        mv2a_t = mv2a_tt[:]
        mv2b_t = mv2b_tt[:]

        xpool0 = ctx.enter_context(tc.tile_pool(name="xp", bufs=8))
        all_Xh = [[None, None] for _ in range(N_CHUNKS)]

        def emit_load(k, eng=None):
            eng = eng or nc.sync
            xflat = x_ap[k * CHUNK : (k + 1) * CHUNK, :].flatten()
            QTR = HALF // 2
            for h in range(2):
                X = xpool0.tile([128, HALF], F32R, tag="X")
                all_Xh[k][h] = X
                for q in range(2):
                    eng.dma_start(
                        X[:, q * QTR : (q + 1) * QTR].rearrange(
                            "p (bh l) -> p bh l", l=128
                        ),
                        xflat[
                            (2 * h + q) * CHUNK * QTR : (2 * h + q + 1) * CHUNK * QTR
                        ].rearrange("(bh p l) -> p bh l", p=128, l=128),
                    )

        # chunk 0, half 0 in eighth-granularity so the first stage-1 group
        # (bh 0-3) can start as early as possible
        xflat0 = x_ap[0:CHUNK, :].flatten()
        EGT = HALF // 4
        X00 = xpool0.tile([128, HALF], F32R, tag="X")
        all_Xh[0][0] = X00
        for e in range(4):
            nc.sync.dma_start(
                X00[:, e * EGT : (e + 1) * EGT].rearrange(
                    "p (bh l) -> p bh l", l=128
                ),
                xflat0[e * CHUNK * EGT : (e + 1) * CHUNK * EGT].rearrange(
                    "(bh p l) -> p bh l", p=128, l=128
                ),
            )
        # chunk 0 half 1 issues from the scalar DGE ring in parallel
        # with sync's chunk-0-half-0 eighths (squares don't need the
        # scalar queue until ~5us in)
        QTR0 = HALF // 2
        X01 = xpool0.tile([128, HALF], F32R, tag="X")
        all_Xh[0][1] = X01
        for q in range(2):
            nc.scalar.dma_start(
                X01[:, q * QTR0 : (q + 1) * QTR0].rearrange(
                    "p (bh l) -> p bh l", l=128
                ),
                xflat0[
                    (2 + q) * CHUNK * QTR0 : (3 + q) * CHUNK * QTR0
                ].rearrange("(bh p l) -> p bh l", p=128, l=128),
            )
        for k in range(1, N_CHUNKS):
            emit_load(k)

        s1pool = ctx.enter_context(tc.tile_pool(name="s1p", bufs=6))
        t12p = ctx.enter_context(tc.tile_pool(name="t12", bufs=4))
        ppool = ctx.enter_context(tc.tile_pool(name="pp", bufs=6))
        ps1 = ctx.enter_context(tc.tile_pool(name="ps1", bufs=2, space="PSUM"))
        ps2 = ctx.enter_context(tc.tile_pool(name="ps2", bufs=2, space="PSUM"))

        # PE clock warmup: the HAM clock gate only grants 2.4 GHz after
        # ~3.4us of sustained activity. These dummy matmuls (no DMA
        # deps) start immediately and burn the cold window during the
        # DMA ramp, so the real matmuls run warm almost from the start.
        warmp = ctx.enter_context(tc.tile_pool(name="warm", bufs=1))
        wt = warmp.tile([128, 256], F32, tag="wt")
        nc.vector.memset(wt[:], 0.0)
        wp = ps2.tile([128, GROUP * 256], F32, tag="g2")
        for _ in range(8):
            nc.tensor.matmul(
                wp[:, 0:256],
                lhsT=wt[:, 0:128].bitcast(F32R),
                rhs=wt[:].bitcast(F32R),
                start=True,
                stop=True,
            )

        # ---- software-pipelined group stream --------------------------
        # One "item" = (k, h, gl). Stage-2+square of item g is emitted
        # after stage-1 of item g+1; adds+store happen per half.
        items = [
            (k, h, gl)
            for k in range(N_CHUNKS)
            for h in range(2)
            for gl in range(4)
        ]
        state = {}  # (k,h) -> dict with T12 tile
        pend = {}  # item -> S1ri tile
        pending_stores = []  # deferred store closures
        LAST = items[-1][:2]

        def emit_half_prologue(k, h):
            T12 = t12p.tile([128, 2 * HALF], BF16, tag="T12")
            P = ppool.tile([128, HALF], F32, tag="P")
            state[(k, h)] = {"T12": T12, "P": P}

        def emit_stage1(item):
            k, h, gl = item
            X = all_Xh[k][h]
            pg = ps1.tile([128, GROUP * 256], F32, tag="g1")
            for j in range(GROUP):
                cl = gl * GROUP + j
                nc.tensor.matmul(
                    pg[:, j * 256 : (j + 1) * 256],
                    lhsT=X[:, cl * 128 : (cl + 1) * 128],
                    rhs=mv1_t,
                    start=True,
                    stop=True,
                )
            # evacuate: two contiguous bf16 casts on VectorE, keeping
            # pg's (j, re/im, q) layout so stage-2 lhsT slices stay
            # contiguous; the split halves the s1->s2 chain latency
            # (stage-2's first matmuls only need the j=0,1 blocks)
            # asymmetric split: a small first piece (the j=0 re/im
            # stationaries) lets stage-2's first matmuls start ~0.9us
            # earlier; the chain latency it cuts sets the pipeline
            # period via the 2-deep psum double buffering
            S1ri = s1pool.tile([128, GROUP * 256], BF16, tag="S1ri")
            nc.vector.tensor_copy(S1ri[:, :256], pg[:, :256])
            nc.vector.tensor_copy(S1ri[:, 256:], pg[:, 256:])
            return S1ri

        def emit_stage2(item, S1ri):
            k, h, gl = item
            st = state[(k, h)]
            pg2 = ps2.tile([128, GROUP * 256], F32, tag="g2")
            for j in range(GROUP):
                nc.tensor.matmul(
                    pg2[:, j * 256 : (j + 1) * 256],
                    lhsT=S1ri[:, j * 256 : j * 256 + 128],
                    rhs=mv2a_t,
                    start=True,
                    stop=False,
                )
                nc.tensor.matmul(
                    pg2[:, j * 256 : (j + 1) * 256],
                    lhsT=S1ri[:, j * 256 + 128 : (j + 1) * 256],
                    rhs=mv2b_t,
                    start=False,
                    stop=True,
                )
            # both squares (re and im) in one ScalarE op (a split was
            # tested and regressed: ps2 recycling is not the binding
            # chain, and the extra op costs ScalarE overhead)
            gcols = slice(gl * GROUP * 256, (gl + 1) * GROUP * 256)
            nc.scalar.square(st["T12"][:, gcols], pg2[:])

        def emit_half_epilogue(k, h):
            st = state.pop((k, h))
            T12 = st["T12"]
            last = (k, h) == LAST
            P = st["P"]
            T5 = T12[:].rearrange(
                "p (g j c q) -> p g j c q", g=4, j=GROUP, c=2
            )
            P4 = P[:].rearrange("p (g j q) -> p g j q", g=4, j=GROUP)
            oflat = out_ap[k * CHUNK : (k + 1) * CHUNK, :].flatten()
            base = h * CHUNK * HALF
            QC = GROUP * 128  # columns per group quarter
            for gl in range(4):
                # adds on GpSimd (otherwise idle); for the final half,
                # alternate so the tail drains in parallel
                eng = (nc.vector if gl % 2 == 0 else nc.gpsimd) if last else nc.gpsimd
                eng.tensor_tensor(
                    P4[:, gl],
                    T5[:, gl, :, 0],
                    T5[:, gl, :, 1],
                    op=mybir.AluOpType.add,
                )
                # all stores on the sync ring: it's free once the loads
                # are issued, and this keeps ScalarE on squares only
                qeng = nc.sync
                qb = base + gl * CHUNK * QC

                def _store(qeng=qeng, qb=qb, P=P, gl=gl, oflat=oflat):
                    qeng.dma_start(
                        oflat[qb : qb + CHUNK * QC].rearrange(
                            "(bh p l) -> p bh l", p=128, l=128
                        ),
                        P[:, gl * QC : (gl + 1) * QC].rearrange(
                            "p (bh l) -> p bh l", l=128
                        ),
                    )

                # defer store issue so the add it depends on has
                # finished by the time it reaches the queue head
                pending_stores.append(_store)
                if len(pending_stores) > 3:
                    pending_stores.pop(0)()

        # depth-2 software pipeline with chunk-boundary flushes: the
        # Tensor queue is in-order, so when the first stage-1 of a new
        # chunk blocks on its DMA, any stage-2 emitted after it is
        # stuck too. Draining the pending stage-2s BEFORE that stage-1
        # banks ~3.5us of matmul work that runs while the data lands.
        s2q = []

        def drain(depth):
            while len(s2q) > depth:
                p2 = s2q.pop(0)
                emit_stage2(p2, pend.pop(p2))
                if p2[2] == 3:
                    emit_half_epilogue(p2[0], p2[1])

        for idx, item in enumerate(items):
            k, h, gl = item
            if idx > 0 and idx % 8 == 0:
                drain(0)  # chunk boundary: flush before the blocking s1
            if gl == 0:
                emit_half_prologue(k, h)
            pend[item] = emit_stage1(item)
            s2q.append(item)
            drain(3)
        drain(0)
        for s in pending_stores:
            s()

    nc.compile()
    return nc


_NC_CACHE = {}


def _get_nc():
    if "nc" not in _NC_CACHE:
        _NC_CACHE["nc"] = _build_nc()
    return _NC_CACHE["nc"]


def kernel(inputs, thetas, phis, lams, _trace=False, _trace_kwargs=None):
    inputs = np.ascontiguousarray(np.asarray(inputs), dtype=np.float32)
    # host-side pre-normalization (per-sample L2 norm), f64 accumulate
    nrm = np.sqrt(
        np.einsum("bi,bi->b", inputs, inputs, dtype=np.float64)
    ).astype(np.float32)
    xn = inputs / nrm[:, None]
    mv1, mv2a, mv2b = _gate_consts(thetas, phis, lams)

    nc = _get_nc()
    in_maps = [
        {
            "x": xn[k * B_CORE : (k + 1) * B_CORE],
            "mv1": mv1,
            "mv2a": mv2a,
            "mv2b": mv2b,
        }
        for k in range(N_CORES)
    ]
    res = run_bass_kernel_spmd(
        nc, in_maps, list(range(N_CORES)), trace=_trace, **(_trace_kwargs or {})
    )
    out = np.concatenate([res.results[k]["probs"] for k in range(N_CORES)], axis=0)
    if _trace:
        kernel.last_result = res
    return out



# revision 2
# speedup vs baseline: 1.0081x; 1.0081x over previous
"""Trainium2 Bass kernel v2 for DifferentiableQuantumCircuit.

Math: output = |U x / ||x|| |^2, U = kron of 12 single-qubit U3 gates x 2
layers; layers fuse into one kron unitary with G_q = U3_l2(q) @ U3_l1(q).
Split: state index i = q5*128 + l7; U = M5a (x) M7b, M5a = kron(G_0..4)
[32x32] on q5, M7b = kron(G_5..11) [128x128] on l7.

v2 changes vs baseline (72.5us):
- bf16 input AND output on the wire (host casts/un-casts; rel_err ~8e-3
  predicted, gate is 2e-2). Halves HBM traffic: 17 MB -> 8.4 MB/core.
- Host pre-permutes x into DMA-native layout: per chunk of 128 samples,
  [128 part=(b2,q5), 1024 cols] contiguous blocks -> 2KB/partition
  descriptor runs (baseline: 512B), 16 loads + 16 stores total.
- Stage 1 in bf16 (was f32r): LDWEIGHTS gets FWL, X tiles half size.
- Stage 2 const-stationary: psum_re/im = Mr^T,Mi^T,-Mi^T stationaries x
  S1re/S1im moving (N=512). Output lands [l7', (j,b2,q5)]-partitioned;
  host un-permutes. 4 MMs/item like baseline but only 3 stationary
  loads, and the S1 evacuation becomes ONE contiguous-dest DVE copy.
- Elementwise per item (16 samples): DVE evac [128,1024] psum->bf16,
  ScalarE square [128,1024] psum->bf16, GpSimd add [128,512] bf16.

Per-item (x32) engine budget: PE ~1.30us, DVE ~1.19us, Scalar ~1.0us,
DMA ~0.72us => PE-bound, ~45us target.
"""

from contextlib import ExitStack

import ml_dtypes
import numpy as np

import concourse.tile as tile
from concourse import bacc, mybir
from concourse.bass_utils import run_bass_kernel_spmd

F32 = mybir.dt.float32
BF16 = mybir.dt.bfloat16
BF = ml_dtypes.bfloat16

NUM_QUBITS = 12
D = 4096
B = 4096
N_CORES = 8
B_CORE = B // N_CORES  # 512
N_CHUNKS = 4  # chunks of 128 samples per core
N_ITEMS = 32  # items of 16 samples (4 c-tiles of 4 samples)
ITEMS_PER_CHUNK = 8


def _u3(theta, phi, lam):
    c = np.cos(theta / 2.0)
    s = np.sin(theta / 2.0)
    return np.array(
        [
            [c, -np.exp(1j * lam) * s],
            [np.exp(1j * phi) * s, np.exp(1j * (phi + lam)) * c],
        ],
        dtype=np.complex128,
    )


def _gate_consts(thetas, phis, lams):
    thetas = np.asarray(thetas, dtype=np.float64)
    phis = np.asarray(phis, dtype=np.float64)
    lams = np.asarray(lams, dtype=np.float64)
    gates = []
    for q in range(NUM_QUBITS):
        g1 = _u3(thetas[0, q], phis[0, q], lams[0, q])
        g2 = _u3(thetas[1, q], phis[1, q], lams[1, q])
        gates.append(g2 @ g1)
    m5a = gates[0]
    for q in range(1, 5):
        m5a = np.kron(m5a, gates[q])  # [32,32] on q5
    m7b = gates[5]
    for q in range(6, 12):
        m7b = np.kron(m7b, gates[q])  # [128,128] on l7

    g5 = np.kron(np.eye(4), m5a)  # block-diag over (b2, q5)
    mv1 = np.concatenate([g5.T.real, g5.T.imag], axis=1)  # [128, 256]

    def bf(a):
        return np.ascontiguousarray(a).astype(BF)

    return (
        bf(mv1),
        bf(m7b.real.T),  # cmr: lhsT with lhsT.T = Re(M7b)
        bf(m7b.imag.T),  # cmi
        bf(-m7b.imag.T),  # cnmi
    )


def _build_nc(num_devices=N_CORES):
    nc = bacc.Bacc(
        "TRN2", target_bir_lowering=False, debug=False, num_devices=num_devices
    )
    # x blocks: row = (k*4 + cb)*128 + (b2*32+q5), col = (bh8, l7)
    x_ap = nc.dram_tensor("x", [16 * 128, 1024], BF16, kind="ExternalInput").ap()
    mv1_ap = nc.dram_tensor("mv1", [128, 256], BF16, kind="ExternalInput").ap()
    cmr_ap = nc.dram_tensor("cmr", [128, 128], BF16, kind="ExternalInput").ap()
    cmi_ap = nc.dram_tensor("cmi", [128, 128], BF16, kind="ExternalInput").ap()
    cnmi_ap = nc.dram_tensor("cnmi", [128, 128], BF16, kind="ExternalInput").ap()
    out_ap = nc.dram_tensor(
        "probs", [16 * 128, 1024], BF16, kind="ExternalOutput"
    ).ap()

    with tile.TileContext(nc) as tc, ExitStack() as ctx:
        # DMA order matters for the ramp: the first x block + mv1 gate
        # s1(0), so they go FIRST on the sync ring; the stage-2 consts
        # (not needed until s2(0), ~3us later) go on the scalar HWDGE
        # ring, which is otherwise idle until the first ACTIVATE.
        consts = ctx.enter_context(tc.tile_pool(name="consts", bufs=1))
        xpool = ctx.enter_context(tc.tile_pool(name="xp", bufs=16))
        xt = []

        X0 = xpool.tile([128, 1024], BF16, tag="X", name="X")
        nc.sync.dma_start(X0[:], x_ap[0:128, :])
        xt.append(X0)
        mv1_t = consts.tile([128, 256], BF16, tag="mv1")
        nc.sync.dma_start(mv1_t[:], mv1_ap[:])
        cmr_t = consts.tile([128, 128], BF16, tag="cmr")
        nc.scalar.dma_start(cmr_t[:], cmr_ap[:])
        cmi_t = consts.tile([128, 128], BF16, tag="cmi")
        nc.scalar.dma_start(cmi_t[:], cmi_ap[:])
        cnmi_t = consts.tile([128, 128], BF16, tag="cnmi")
        nc.scalar.dma_start(cnmi_t[:], cnmi_ap[:])

        for b in range(1, 16):
            X = xpool.tile([128, 1024], BF16, tag="X", name="X")
            nc.sync.dma_start(X[:], x_ap[b * 128 : (b + 1) * 128, :])
            xt.append(X)

        s1pool = ctx.enter_context(tc.tile_pool(name="s1p", bufs=4))
        sqpool = ctx.enter_context(tc.tile_pool(name="sqp", bufs=3))
        ppool = ctx.enter_context(tc.tile_pool(name="pp", bufs=3))
        ps1 = ctx.enter_context(tc.tile_pool(name="ps1", bufs=2, space="PSUM"))
        ps2 = ctx.enter_context(tc.tile_pool(name="ps2", bufs=2, space="PSUM"))

        # PE clock warmup: dummy matmuls with no DMA deps burn the cold
        # HAM window (~3.4us) while the first x blocks land.
        warmp = ctx.enter_context(tc.tile_pool(name="warm", bufs=1))
        wt = warmp.tile([128, 256], BF16, tag="wt")
        # gpsimd's preamble ends earliest, so memset there -> warmup MMs
        # can issue sooner. Only 6 warmup MMs: they just need to bridge
        # until the first x block lands (~8us); real items continue the
        # HAM-warming busy window and do useful work while cold.
        nc.gpsimd.memset(wt[:], 0.0)
        wp = ps2.tile([128, 1024], F32, tag="ps2")
        for _ in range(6):
            nc.tensor.matmul(
                wp[:, 0:256],
                lhsT=wt[:, 0:128],
                rhs=wt[:],
                start=True,
                stop=True,
            )

        # ---- software-pipelined item stream ---------------------------
        # item i (16 samples): c-tiles cl = 4i..4i+3 of chunk k=i//8,
        # living in x block b = (k*4 + (i%8)//2) at col offset (i%2)*512.
        def emit_s1(i):
            k, it = divmod(i, ITEMS_PER_CHUNK)
            X = xt[k * 4 + it // 2]
            off = (it % 2) * 512
            pg = ps1.tile([128, 1024], F32, tag="ps1")
            for j in range(4):
                nc.tensor.matmul(
                    pg[:, j * 256 : (j + 1) * 256],
                    lhsT=X[:, off + j * 128 : off + (j + 1) * 128],
                    rhs=mv1_t[:],
                    start=True,
                    stop=True,
                )
            return pg

        def emit_evac(pg):
            # contiguous psum->sbuf bf16 cast; S1 keeps pg's (j, reim, c)
            # column order, stage-2 rhs uses strided views instead.
            S1 = s1pool.tile([128, 1024], BF16, tag="S1")
            nc.vector.tensor_copy(S1[:], pg[:])
            return S1

        def emit_s2(S1):
            pg2 = ps2.tile([128, 1024], F32, tag="ps2")
            sv = S1[:].rearrange("p (j r c) -> p r j c", j=4, r=2)
            s1re, s1im = sv[:, 0], sv[:, 1]  # [p, 4, 128] strided views
            psre, psim = pg2[:, 0:512], pg2[:, 512:1024]
            # psum_re = Mr@re - Mi@im ; psum_im = Mr@im + Mi@re
            # (nested accumulation groups; interleaving them hard-faulted
            # the exec unit on HW even though CoreSim accepted it)
            nc.tensor.matmul(psre, lhsT=cmr_t[:], rhs=s1re, start=True, stop=False)
            nc.tensor.matmul(psre, lhsT=cnmi_t[:], rhs=s1im, start=False, stop=True)
            nc.tensor.matmul(psim, lhsT=cmi_t[:], rhs=s1re, start=True, stop=False)
            nc.tensor.matmul(psim, lhsT=cmr_t[:], rhs=s1im, start=False, stop=True)
            return pg2

        def emit_sq(pg2):
            SQ = sqpool.tile([128, 1024], BF16, tag="SQ")
            nc.scalar.square(SQ[:], pg2[:])
            return SQ

        P_cur = [None]

        def emit_add_store(i, SQ):
            # last two items: per-item add + store so the tail drains
            # fast after the final matmul (item 31's add on DVE, which
            # is idle by then; bf16-sbuf TT runs 2x there)
            if i >= N_ITEMS - 2:
                P = ppool.tile([128, 512], BF16, tag="P", name="P")
                eng = nc.vector if i == N_ITEMS - 1 else nc.gpsimd
                eng.tensor_tensor(
                    P[:], SQ[:, 0:512], SQ[:, 512:1024], op=mybir.AluOpType.add
                )
                b, half = divmod(i, 2)
                nc.sync.dma_start(
                    out_ap[b * 128 : (b + 1) * 128, half * 512 : (half + 1) * 512],
                    P[:],
                )
                return
            half = i % 2
            if half == 0:
                P_cur[0] = ppool.tile([128, 1024], BF16, tag="P", name="P")
            P = P_cur[0]
            nc.gpsimd.tensor_tensor(
                P[:, half * 512 : (half + 1) * 512],
                SQ[:, 0:512],
                SQ[:, 512:1024],
                op=mybir.AluOpType.add,
            )
            if half == 1:
                b = i // 2  # output block 0..15
                nc.sync.dma_start(out_ap[b * 128 : (b + 1) * 128, :], P[:])

        # depth-2 pipeline on the in-order PE queue:
        # s1(0), s1(1), s1(2), s2(0), s1(3), s2(1), ...
        pend = {}
        pend[0] = emit_evac(emit_s1(0))
        pend[1] = emit_evac(emit_s1(1))
        for i in range(N_ITEMS):
            if i + 2 < N_ITEMS:
                pend[i + 2] = emit_evac(emit_s1(i + 2))
            S1 = pend.pop(i)
            SQ = emit_sq(emit_s2(S1))
            emit_add_store(i, SQ)

    nc.compile()
    return nc


_NC_CACHE = {}


def _get_nc():
    if "nc" not in _NC_CACHE:
        _NC_CACHE["nc"] = _build_nc()
    return _NC_CACHE["nc"]


def kernel(inputs, thetas, phis, lams, _trace=False, _trace_kwargs=None):
    inputs = np.ascontiguousarray(np.asarray(inputs), dtype=np.float32)
    nrm = np.sqrt(
        np.einsum("bi,bi->b", inputs, inputs, dtype=np.float64)
    ).astype(np.float32)
    xn = inputs / nrm[:, None]

    # host permute: [core, k, cb, bh8, b2, q5, l7] -> [core, k, cb, (b2,q5), (bh8,l7)]
    xp = xn.reshape(N_CORES, N_CHUNKS, 4, 8, 4, 32, 128)
    xp = np.ascontiguousarray(xp.transpose(0, 1, 2, 4, 5, 3, 6)).astype(BF)
    xp = xp.reshape(N_CORES, 16 * 128, 1024)

    mv1, cmr, cmi, cnmi = _gate_consts(thetas, phis, lams)

    nc = _get_nc()
    in_maps = [
        {"x": xp[c], "mv1": mv1, "cmr": cmr, "cmi": cmi, "cnmi": cnmi}
        for c in range(N_CORES)
    ]
    res = run_bass_kernel_spmd(
        nc, in_maps, list(range(N_CORES)), trace=_trace, **(_trace_kwargs or {})
    )
    # out blocks: [core][k, ih, l7', (it2, j, b2, q5)] ; s = k*128+bh*4+b2,
    # bh = (ih*2+it2)*4 + j, i = q5*128 + l7'
    out = np.stack([res.results[c]["probs"] for c in range(N_CORES)], axis=0)
    out = out.reshape(N_CORES, N_CHUNKS, 4, 128, 2, 4, 4, 32)
    out = out.transpose(0, 1, 2, 4, 5, 6, 7, 3)  # -> [c,k,ih,it2,j,b2,q5,l7']
    out = np.ascontiguousarray(out).reshape(B, D).astype(np.float32)
    if _trace:
        kernel.last_result = res
    return out


# revision 3
# speedup vs baseline: 1.0254x; 1.0172x over previous
"""Trainium2 Bass kernel v2 for DifferentiableQuantumCircuit.

Math: output = |U x / ||x|| |^2, U = kron of 12 single-qubit U3 gates x 2
layers; layers fuse into one kron unitary with G_q = U3_l2(q) @ U3_l1(q).
Split: state index i = q5*128 + l7; U = M5a (x) M7b, M5a = kron(G_0..4)
[32x32] on q5, M7b = kron(G_5..11) [128x128] on l7.

v2 changes vs baseline (72.5us):
- bf16 input AND output on the wire (host casts/un-casts; rel_err ~8e-3
  predicted, gate is 2e-2). Halves HBM traffic: 17 MB -> 8.4 MB/core.
- Host pre-permutes x into DMA-native layout: per chunk of 128 samples,
  [128 part=(b2,q5), 1024 cols] contiguous blocks -> 2KB/partition
  descriptor runs (baseline: 512B), 16 loads + 16 stores total.
- Stage 1 in bf16 (was f32r): LDWEIGHTS gets FWL, X tiles half size.
- Stage 2 const-stationary: psum_re/im = Mr^T,Mi^T,-Mi^T stationaries x
  S1re/S1im moving (N=512). Output lands [l7', (j,b2,q5)]-partitioned;
  host un-permutes. 4 MMs/item like baseline but only 3 stationary
  loads, and the S1 evacuation becomes ONE contiguous-dest DVE copy.
- Elementwise per item (16 samples): DVE evac [128,1024] psum->bf16,
  ScalarE square [128,1024] psum->bf16, GpSimd add [128,512] bf16.

Per-item (x32) engine budget: PE ~1.30us, DVE ~1.19us, Scalar ~1.0us,
DMA ~0.72us => PE-bound, ~45us target.
"""

from contextlib import ExitStack

import ml_dtypes
import numpy as np

import concourse.tile as tile
from concourse import bacc, mybir
from concourse.bass_utils import run_bass_kernel_spmd

F32 = mybir.dt.float32
BF16 = mybir.dt.bfloat16
BF = ml_dtypes.bfloat16

NUM_QUBITS = 12
D = 4096
B = 4096
N_CORES = 8
B_CORE = B // N_CORES  # 512
N_CHUNKS = 4  # chunks of 128 samples per core
N_ITEMS = 32  # items of 16 samples (4 c-tiles of 4 samples)
ITEMS_PER_CHUNK = 8


def _u3(theta, phi, lam):
    c = np.cos(theta / 2.0)
    s = np.sin(theta / 2.0)
    return np.array(
        [
            [c, -np.exp(1j * lam) * s],
            [np.exp(1j * phi) * s, np.exp(1j * (phi + lam)) * c],
        ],
        dtype=np.complex128,
    )


def _gate_consts(thetas, phis, lams):
    thetas = np.asarray(thetas, dtype=np.float64)
    phis = np.asarray(phis, dtype=np.float64)
    lams = np.asarray(lams, dtype=np.float64)
    gates = []
    for q in range(NUM_QUBITS):
        g1 = _u3(thetas[0, q], phis[0, q], lams[0, q])
        g2 = _u3(thetas[1, q], phis[1, q], lams[1, q])
        gates.append(g2 @ g1)
    m5a = gates[0]
    for q in range(1, 5):
        m5a = np.kron(m5a, gates[q])  # [32,32] on q5
    m7b = gates[5]
    for q in range(6, 12):
        m7b = np.kron(m7b, gates[q])  # [128,128] on l7

    g5 = np.kron(np.eye(4), m5a)  # block-diag over (b2, q5)
    mv1 = np.concatenate([g5.T.real, g5.T.imag], axis=1)  # [128, 256]

    def bf(a):
        return np.ascontiguousarray(a).astype(BF)

    return (
        bf(mv1),
        bf(m7b.real.T),  # cmr: lhsT with lhsT.T = Re(M7b)
        bf(m7b.imag.T),  # cmi
        bf(-m7b.imag.T),  # cnmi
    )


def _build_nc(num_devices=N_CORES):
    nc = bacc.Bacc(
        "TRN2", target_bir_lowering=False, debug=False, num_devices=num_devices
    )
    # x blocks: row = (k*4 + cb)*128 + (b2*32+q5), col = (bh8, l7)
    x_ap = nc.dram_tensor("x", [16 * 128, 1024], BF16, kind="ExternalInput").ap()
    mv1_ap = nc.dram_tensor("mv1", [128, 256], BF16, kind="ExternalInput").ap()
    cmr_ap = nc.dram_tensor("cmr", [128, 128], BF16, kind="ExternalInput").ap()
    cmi_ap = nc.dram_tensor("cmi", [128, 128], BF16, kind="ExternalInput").ap()
    cnmi_ap = nc.dram_tensor("cnmi", [128, 128], BF16, kind="ExternalInput").ap()
    out_ap = nc.dram_tensor(
        "probs", [16 * 128, 1024], BF16, kind="ExternalOutput"
    ).ap()

    with tile.TileContext(nc) as tc, ExitStack() as ctx:
        # DMA order matters for the ramp: the first x block + mv1 gate
        # s1(0), so they go FIRST on the sync ring; the stage-2 consts
        # (not needed until s2(0), ~3us later) go on the scalar HWDGE
        # ring, which is otherwise idle until the first ACTIVATE.
        consts = ctx.enter_context(tc.tile_pool(name="consts", bufs=1))
        xpool = ctx.enter_context(tc.tile_pool(name="xp", bufs=16))
        xt = []

        X0 = xpool.tile([128, 1024], BF16, tag="X", name="X")
        nc.sync.dma_start(X0[:], x_ap[0:128, :])
        xt.append(X0)
        mv1_t = consts.tile([128, 256], BF16, tag="mv1")
        nc.sync.dma_start(mv1_t[:], mv1_ap[:])
        cmr_t = consts.tile([128, 128], BF16, tag="cmr")
        nc.scalar.dma_start(cmr_t[:], cmr_ap[:])
        cmi_t = consts.tile([128, 128], BF16, tag="cmi")
        nc.scalar.dma_start(cmi_t[:], cmi_ap[:])
        cnmi_t = consts.tile([128, 128], BF16, tag="cnmi")
        nc.scalar.dma_start(cnmi_t[:], cnmi_ap[:])

        for b in range(1, 16):
            X = xpool.tile([128, 1024], BF16, tag="X", name="X")
            nc.sync.dma_start(X[:], x_ap[b * 128 : (b + 1) * 128, :])
            xt.append(X)

        s1pool = ctx.enter_context(tc.tile_pool(name="s1p", bufs=4))
        sqpool = ctx.enter_context(tc.tile_pool(name="sqp", bufs=3))
        ppool = ctx.enter_context(tc.tile_pool(name="pp", bufs=3))
        ps1 = ctx.enter_context(tc.tile_pool(name="ps1", bufs=2, space="PSUM"))
        ps2 = ctx.enter_context(tc.tile_pool(name="ps2", bufs=2, space="PSUM"))

        # PE clock warmup: dummy matmuls with no DMA deps burn the cold
        # HAM window (~3.4us) while the first x blocks land.
        warmp = ctx.enter_context(tc.tile_pool(name="warm", bufs=1))
        wt = warmp.tile([128, 256], BF16, tag="wt")
        # gpsimd's preamble ends earliest, so memset there -> warmup MMs
        # can issue sooner. Only 6 warmup MMs: they just need to bridge
        # until the first x block lands (~8us); real items continue the
        # HAM-warming busy window and do useful work while cold.
        nc.gpsimd.memset(wt[:], 0.0)
        wp = ps2.tile([128, 1024], F32, tag="ps2")
        # enough cold-clock warmup (~3.4us) to bridge the X0 DMA wait
        # with NO PE-idle gap, so HAM un-throttles right as the real
        # items begin instead of ~3us into them
        for _ in range(16):
            nc.tensor.matmul(
                wp[:, 0:256],
                lhsT=wt[:, 0:128],
                rhs=wt[:],
                start=True,
                stop=True,
            )

        # ---- software-pipelined item stream ---------------------------
        # item i (16 samples): c-tiles cl = 4i..4i+3 of chunk k=i//8,
        # living in x block b = (k*4 + (i%8)//2) at col offset (i%2)*512.
        def emit_s1(i):
            k, it = divmod(i, ITEMS_PER_CHUNK)
            X = xt[k * 4 + it // 2]
            off = (it % 2) * 512
            pg = ps1.tile([128, 1024], F32, tag="ps1")
            for j in range(4):
                nc.tensor.matmul(
                    pg[:, j * 256 : (j + 1) * 256],
                    lhsT=X[:, off + j * 128 : off + (j + 1) * 128],
                    rhs=mv1_t[:],
                    start=True,
                    stop=True,
                )
            return pg

        def emit_evac(pg):
            # contiguous psum->sbuf bf16 cast; S1 keeps pg's (j, reim, c)
            # column order, stage-2 rhs uses strided views instead.
            S1 = s1pool.tile([128, 1024], BF16, tag="S1")
            nc.vector.tensor_copy(S1[:], pg[:])
            return S1

        def emit_s2(S1):
            pg2 = ps2.tile([128, 1024], F32, tag="ps2")
            sv = S1[:].rearrange("p (j r c) -> p r j c", j=4, r=2)
            s1re, s1im = sv[:, 0], sv[:, 1]  # [p, 4, 128] strided views
            psre, psim = pg2[:, 0:512], pg2[:, 512:1024]
            # psum_re = Mr@re - Mi@im ; psum_im = Mr@im + Mi@re
            # (nested accumulation groups; interleaving them hard-faulted
            # the exec unit on HW even though CoreSim accepted it)
            nc.tensor.matmul(psre, lhsT=cmr_t[:], rhs=s1re, start=True, stop=False)
            nc.tensor.matmul(psre, lhsT=cnmi_t[:], rhs=s1im, start=False, stop=True)
            nc.tensor.matmul(psim, lhsT=cmi_t[:], rhs=s1re, start=True, stop=False)
            nc.tensor.matmul(psim, lhsT=cmr_t[:], rhs=s1im, start=False, stop=True)
            return pg2

        def emit_sq(pg2):
            SQ = sqpool.tile([128, 1024], BF16, tag="SQ")
            nc.scalar.square(SQ[:], pg2[:])
            return SQ

        P_cur = [None]

        def emit_add_store(i, SQ):
            # last two items: per-item add + store so the tail drains
            # fast after the final matmul (item 31's add on DVE, which
            # is idle by then; bf16-sbuf TT runs 2x there)
            if i >= N_ITEMS - 2:
                P = ppool.tile([128, 512], BF16, tag="P", name="P")
                eng = nc.vector if i == N_ITEMS - 1 else nc.gpsimd
                eng.tensor_tensor(
                    P[:], SQ[:, 0:512], SQ[:, 512:1024], op=mybir.AluOpType.add
                )
                b, half = divmod(i, 2)
                nc.sync.dma_start(
                    out_ap[b * 128 : (b + 1) * 128, half * 512 : (half + 1) * 512],
                    P[:],
                )
                return
            half = i % 2
            if half == 0:
                P_cur[0] = ppool.tile([128, 1024], BF16, tag="P", name="P")
            P = P_cur[0]
            nc.gpsimd.tensor_tensor(
                P[:, half * 512 : (half + 1) * 512],
                SQ[:, 0:512],
                SQ[:, 512:1024],
                op=mybir.AluOpType.add,
            )
            if half == 1:
                b = i // 2  # output block 0..15
                nc.sync.dma_start(out_ap[b * 128 : (b + 1) * 128, :], P[:])

        # depth-2 pipeline on the in-order PE queue:
        # s1(0), s1(1), s1(2), s2(0), s1(3), s2(1), ...
        pend = {}
        pend[0] = emit_evac(emit_s1(0))
        pend[1] = emit_evac(emit_s1(1))
        for i in range(N_ITEMS):
            if i + 2 < N_ITEMS:
                pend[i + 2] = emit_evac(emit_s1(i + 2))
            S1 = pend.pop(i)
            SQ = emit_sq(emit_s2(S1))
            emit_add_store(i, SQ)

    nc.compile()
    return nc


_NC_CACHE = {}


def _get_nc():
    if "nc" not in _NC_CACHE:
        _NC_CACHE["nc"] = _build_nc()
    return _NC_CACHE["nc"]


def kernel(inputs, thetas, phis, lams, _trace=False, _trace_kwargs=None):
    inputs = np.ascontiguousarray(np.asarray(inputs), dtype=np.float32)
    nrm = np.sqrt(
        np.einsum("bi,bi->b", inputs, inputs, dtype=np.float64)
    ).astype(np.float32)
    xn = inputs / nrm[:, None]

    # host permute: [core, k, cb, bh8, b2, q5, l7] -> [core, k, cb, (b2,q5), (bh8,l7)]
    xp = xn.reshape(N_CORES, N_CHUNKS, 4, 8, 4, 32, 128)
    xp = np.ascontiguousarray(xp.transpose(0, 1, 2, 4, 5, 3, 6)).astype(BF)
    xp = xp.reshape(N_CORES, 16 * 128, 1024)

    mv1, cmr, cmi, cnmi = _gate_consts(thetas, phis, lams)

    nc = _get_nc()
    in_maps = [
        {"x": xp[c], "mv1": mv1, "cmr": cmr, "cmi": cmi, "cnmi": cnmi}
        for c in range(N_CORES)
    ]
    res = run_bass_kernel_spmd(
        nc, in_maps, list(range(N_CORES)), trace=_trace, **(_trace_kwargs or {})
    )
    # out blocks: [core][k, ih, l7', (it2, j, b2, q5)] ; s = k*128+bh*4+b2,
    # bh = (ih*2+it2)*4 + j, i = q5*128 + l7'
    out = np.stack([res.results[c]["probs"] for c in range(N_CORES)], axis=0)
    out = out.reshape(N_CORES, N_CHUNKS, 4, 128, 2, 4, 4, 32)
    out = out.transpose(0, 1, 2, 4, 5, 6, 7, 3)  # -> [c,k,ih,it2,j,b2,q5,l7']
    out = np.ascontiguousarray(out).reshape(B, D).astype(np.float32)
    if _trace:
        kernel.last_result = res
    return out
